# revision 1
# baseline (speedup 1.0000x reference)
"""Trainium2 Bass kernel for the CRF forward algorithm (nn_CRF).

Reference computes: scan over S=8192 steps of
    fv'[i] = logsumexp_j(fv[j] + transitions[i, j]) + h[s, i]
then logsumexp(fv + transitions[END_IDX]).

Algorithm used here (exp-space scan with exact running normalizer):
    W = exp(transitions)            (computed on device, bf16)
    v = fv - C   (normalized state, [2048] fp32)
    per step:
        w   = exp(v)                           (bf16)
        E   = W @ w                            (PE matvec, fp32 psum)
        mh  = ln(colsum . w)  = ln(sum_i E_i)  (PE skinny matmul + ACT Ln)
        v'  = ln(E + 1e-30) + h[s] - mh
        C  += mh
    answer = C + ln(sum_i exp(v_i + transitions[END_IDX, i]))
mh = ln(sum_i E_i) >= max_i ln(E_i), and <= max + ln(2048), so max(v) stays
bounded in ~[-13, +6]: exp never overflows and dominant terms never flush.
mh cancels exactly between C and v', so only its magnitude matters.
colsum[j] = sum_i exp(tr[i,j]) is precomputed on device.

Layout: tag j -> (slot k = j//128, partition p = j%128); v/w/h tiles are
[128, 16].  W^T lives in SBUF as 256 bf16 tiles [128 j, 128 i], tile
t = k*16 + g at free offset t*128 (k = j-slot, g = i-group).

Each of the 8 cores runs the identical full recurrence (replicated); core 0's
output is returned.  (Cross-core remote-DMA crashes this runtime and in-loop
collectives are architecturally impossible, so replication is the reliable
distribution.)
"""
import sys

sys.path.insert(0, "/opt/trn_rl_repo")

import numpy as np

S = 8192
T = 2048
P = 128
NSLOT = T // P          # 16 j-slots
NGRP = T // P           # 16 i-groups
NBLK = NSLOT * NGRP     # 256 W tiles
UNROLL = 2              # steps per loop iteration (h double-buffer parity)
EPS = 1e-30


def build_kernel(n_steps=S, hsb_rows=None, timing_mode=False):
    import concourse.bacc as bacc
    import concourse.bass as bass
    import concourse.mybir as mybir
    from contextlib import ExitStack

    if timing_mode:
        hsb_rows = 2
    hsb_rows = hsb_rows or n_steps
    assert n_steps % UNROLL == 0
    fp32 = mybir.dt.float32
    bf16 = mybir.dt.bfloat16
    AF = mybir.ActivationFunctionType
    ALU = mybir.AluOpType
    AX = mybir.AxisListType

    nc = bacc.Bacc("TRN2", target_bir_lowering=True, num_devices=8)

    n_wtb = 2 if timing_mode else NBLK
    wtb = nc.declare_dram_parameter("wtb", [n_wtb, P, P], fp32, isOutput=False)
    hsb = nc.declare_dram_parameter("hsb", [hsb_rows, T], fp32, isOutput=False)
    v0f = nc.declare_dram_parameter("v0f", [P, NSLOT], fp32, isOutput=False)
    trendf = nc.declare_dram_parameter("trendf", [P, NSLOT], fp32, isOutput=False)
    out_d = nc.declare_dram_parameter("out", [1, 1], fp32, isOutput=True)

    ctx = ExitStack()
    sb = lambda name, shape, dt: ctx.enter_context(nc.sbuf_tensor(name, shape, dt))
    ps = lambda name, shape, dt: ctx.enter_context(nc.psum_tensor(name, shape, dt))
    sem = lambda name: ctx.enter_context(nc.semaphore(name))

    with ctx:
        wt = sb("wt", [P, NBLK * P], bf16)   # W^T, 256 tiles of [128,128]
        colsum = sb("colsum", [P, NSLOT], fp32)
        colsum_bf = sb("colsum_bf", [P, NSLOT], bf16)
        v = sb("v", [P, NSLOT], fp32)
        w = sb("w", [P, NSLOT], bf16)
        ln_out = sb("ln_out", [P, NSLOT], fp32)
        es = sb("es", [P, NSLOT], fp32)      # h[s] - mh ; reused at the end
        h_step = [sb(f"h_step{i}", [P, NSLOT], fp32) for i in range(UNROLL)]
        tmp = [sb(f"tmp{i}", [P, P], fp32) for i in range(2)]
        ones_col = sb("ones_col", [P, 1], fp32)
        eps_t = sb("eps_t", [P, 1], fp32)
        ones_row = sb("ones_row", [1, P], fp32)
        m_sb = sb("m_sb", [1, 1], fp32)      # mh scalar
        c_acc = sb("c_acc", [1, 1], fp32)    # C accumulator
        trend = sb("trend", [P, NSLOT], fp32)
        fin = sb("fin", [1, 1], fp32)

        psum_mv = ps("psum_mv", [P, NSLOT], fp32)
        psum_m = ps("psum_m", [1, 1], fp32)
        psum_b = ps("psum_b", [P, 1], fp32)
        psum_f = ps("psum_f", [1, 1], fp32)

        su_dma = [sem("su_dma0"), sem("su_dma1")]  # wtb DMAs by parity
        su_exp = sem("su_exp")       # setup exp done (+1 per block)
        su_misc = sem("su_misc")     # consts / v0 / trend ready
        h_ready = [sem("h_ready0"), sem("h_ready1")]  # h DMA by parity
        w_sem = sem("w_sem")         # ACT exp done (+1 per step)
        pe1 = sem("pe1")             # PE mv+skinny done (+1 per step)
        pe2 = sem("pe2")             # PE mh-bcast done (+1 per step)
        act_ln = sem("act_ln")       # ACT Ln pair done (+1 per step)
        dve_st = sem("dve_st")       # DVE step done (+1 per step)
        fin_sem = sem("fin_sem")

        n_iter = n_steps // UNROLL

        with nc.Block() as block:

            # ---------------- sync engine: all input DMAs ----------------
            @block.sync
            def _(eng):
                eng.dma_start(v[:, :], v0f[:, :]).then_inc(su_misc, 16)
                eng.dma_start(trend[:, :], trendf[:, :]).then_inc(su_misc, 16)
                for t in range(NBLK):
                    if t >= 2:
                        eng.wait_ge(su_exp, t - 1)
                    eng.dma_start(
                        tmp[t % 2][:, :],
                        wtb[(t % 2 if timing_mode else t), :, :],
                    ).then_inc(su_dma[t % 2], 16)
                # h prologue: steps 0..UNROLL-1
                for s in range(UNROLL):
                    eng.dma_start(
                        h_step[s][:, :],
                        hsb[(0 if timing_mode else s) : (1 if timing_mode else s + 1), :],
                    ).then_inc(h_ready[s % 2], 16)
                r_off = eng.alloc_register("r_off")   # step index
                r_g = eng.alloc_register("r_g")       # dve_st guard
                r_i = eng.alloc_register("r_i")
                eng.reg_mov(r_off, 0 if timing_mode else UNROLL)
                eng.reg_mov(r_g, 0)
                eng.reg_mov(r_i, 0)
                eng.br("sync_loop")
                with nc.body("sync_loop"):
                    for u in range(UNROLL):
                        eng.reg_add(r_g, r_g, 1)
                        eng.wait_ge(dve_st, r_g)
                        eng.dma_start(
                            h_step[u][:, :],
                            hsb[bass.ds(eng.snap(r_off), 1), :],
                        ).then_inc(h_ready[u % 2], 16)
                        if not timing_mode:
                            eng.reg_add(r_off, r_off, 1)
                    eng.reg_add(r_i, r_i, 1)
                    eng.br_lt(r_i, n_iter - 1, "sync_loop", "sync_done")
                with nc.body("sync_done"):
                    eng.wait_ge(fin_sem, 5)
                    eng.dma_start(out_d[:, :], fin[:, :]).then_inc(su_misc, 16)
                    eng.br(block.end_bb)

            # ---------------- gpsimd: constants only ----------------
            @block.gpsimd
            def _(eng):
                eng.memset(ones_col[:, :], 1.0)
                eng.memset(eps_t[:, :], EPS)
                eng.memset(ones_row[:, :], 1.0)
                eng.memset(c_acc[:, :], 0.0)
                eng.drain()
                eng.nop().then_inc(su_misc, 16)

            # ------------- scalar (ACT): W exp setup, loop exp/ln ----------
            @block.scalar
            def _(eng):
                for t in range(NBLK):
                    eng.wait_ge(su_dma[t % 2], 16 * (t // 2 + 1))
                    eng.activation(
                        wt[:, t * P : (t + 1) * P], tmp[t % 2][:, :], AF.Exp
                    ).then_inc(su_exp, 1)
                r_v = eng.alloc_register("r_v")    # dve_st target
                r_pe = eng.alloc_register("r_pe")  # pe1 target
                r_i = eng.alloc_register("r_i")
                eng.reg_mov(r_v, 0)
                eng.reg_mov(r_pe, 0)
                eng.reg_mov(r_i, 0)
                eng.wait_ge(su_misc, 48)
                eng.br("act_loop")
                with nc.body("act_loop"):
                    for u in range(UNROLL):
                        eng.wait_ge(dve_st, r_v)      # v from prev step
                        eng.wait_ge(pe1, r_pe)        # w free (prev matvec)
                        eng.activation(w[:, :], v[:, :], AF.Exp).then_inc(
                            w_sem, 1
                        )
                        eng.reg_add(r_pe, r_pe, 1)
                        eng.wait_ge(pe1, r_pe)        # this step's matvec done
                        eng.activation(
                            ln_out[:, :], psum_mv[:, :], AF.Ln,
                            bias=eps_t[:, :],
                        )
                        eng.activation(m_sb[:, :], psum_m[:, :], AF.Ln).then_inc(
                            act_ln, 1
                        )
                        eng.reg_add(r_v, r_v, 1)
                    eng.reg_add(r_i, r_i, 1)
                    eng.br_lt(r_i, n_iter, "act_loop", "act_fin")
                with nc.body("act_fin"):
                    eng.wait_ge(fin_sem, 1)
                    eng.activation(ln_out[:, :], es[:, :], AF.Exp).then_inc(
                        fin_sem, 1
                    )
                    eng.wait_ge(pe2, n_steps + 1)
                    eng.activation(m_sb[:, :], psum_f[:, :], AF.Ln).then_inc(
                        fin_sem, 1
                    )
                    eng.br(block.end_bb)

            # ------------- tensor (PE): matvec + skinny + bcast -------------
            @block.tensor
            def _(eng):
                r_w = eng.alloc_register("r_w")
                r_ln = eng.alloc_register("r_ln")
                r_dve = eng.alloc_register("r_dve")
                r_i = eng.alloc_register("r_i")
                eng.reg_mov(r_w, 0)
                eng.reg_mov(r_ln, 0)
                eng.reg_mov(r_dve, 0)
                eng.reg_mov(r_i, 0)
                eng.wait_ge(su_misc, 64)
                eng.br("pe_loop")
                with nc.body("pe_loop"):
                    for u in range(UNROLL):
                        eng.reg_add(r_w, r_w, 1)
                        eng.wait_ge(w_sem, r_w)       # w ready
                        eng.wait_ge(act_ln, r_ln)     # psum_mv/m free
                        eng.wait_ge(dve_st, r_dve)    # psum_b free
                        for g in range(NGRP):
                            for k in range(NSLOT):
                                t = k * NGRP + g
                                eng.matmul(
                                    psum_mv[:, g : g + 1],
                                    wt[:, t * P : (t + 1) * P],
                                    w[:, k : k + 1],
                                    start=(k == 0),
                                    stop=(k == NSLOT - 1),
                                )
                        for k in range(NSLOT):
                            mm = eng.matmul(
                                psum_m[:, :],
                                colsum_bf[:, k : k + 1],
                                w[:, k : k + 1],
                                start=(k == 0),
                                stop=(k == NSLOT - 1),
                            )
                            if k == NSLOT - 1:
                                mm.then_inc(pe1, 1)
                        eng.reg_add(r_ln, r_ln, 1)
                        eng.wait_ge(act_ln, r_ln)     # mh ready
                        eng.matmul(
                            psum_b[:, :],
                            ones_row[:, :],
                            m_sb[:, :],
                            start=True,
                            stop=True,
                        ).then_inc(pe2, 1)
                        eng.reg_add(r_dve, r_dve, 1)
                    eng.reg_add(r_i, r_i, 1)
                    eng.br_lt(r_i, n_iter, "pe_loop", "pe_fin")
                with nc.body("pe_fin"):
                    eng.wait_ge(fin_sem, 3)
                    eng.matmul(
                        psum_f[:, :],
                        es[:, 0:1],
                        ones_col[:, :],
                        start=True,
                        stop=True,
                    ).then_inc(pe2, 1)
                    eng.br(block.end_bb)

            # ------------- vector (DVE): colsum setup + per-step tail -------
            @block.vector
            def _(eng):
                for k in range(NSLOT):
                    eng.wait_ge(su_exp, (k + 1) * NGRP)
                    eng.tensor_reduce(
                        colsum[:, k : k + 1],
                        wt[:, k * NGRP * P : (k + 1) * NGRP * P],
                        axis=AX.X,
                        op=ALU.add,
                    )
                eng.drain()
                eng.tensor_copy(colsum_bf[:, :], colsum[:, :]).then_inc(
                    su_misc, 16
                )
                r_pe2 = eng.alloc_register("r_pe2")
                r_ln = eng.alloc_register("r_ln")
                r_h = eng.alloc_register("r_h")
                r_wr = eng.alloc_register("r_wr")
                r_i = eng.alloc_register("r_i")
                eng.reg_mov(r_pe2, 0)
                eng.reg_mov(r_ln, 0)
                eng.reg_mov(r_h, 0)
                eng.reg_mov(r_wr, 0)
                eng.reg_mov(r_i, 0)
                eng.wait_ge(su_misc, 48)
                eng.br("dve_loop")
                with nc.body("dve_loop"):
                    for u in range(UNROLL):
                        eng.reg_add(r_pe2, r_pe2, 1)
                        eng.reg_add(r_ln, r_ln, 1)
                        if u == 0:
                            eng.reg_add(r_h, r_h, 16)
                        eng.reg_add(r_wr, r_wr, 1)
                        eng.wait_ge(h_ready[u % 2], r_h)
                        eng.wait_ge(pe2, r_pe2)       # psum_b (mh bcast)
                        eng.drain()                   # es WAR vs prev v-add
                        eng.tensor_scalar(
                            es[:, :],
                            h_step[u][:, :],
                            psum_b[:, :],
                            None,
                            op0=ALU.subtract,
                        )
                        eng.tensor_tensor(
                            c_acc[:, :], c_acc[:, :], m_sb[:, :], op=ALU.add
                        )
                        eng.drain()                   # es RAW
                        eng.wait_ge(act_ln, r_ln)     # ln_out ready
                        eng.wait_ge(w_sem, r_wr)      # exp done reading v
                        eng.tensor_tensor(
                            v[:, :], ln_out[:, :], es[:, :], op=ALU.add
                        ).then_inc(dve_st, 1)
                    eng.reg_add(r_i, r_i, 1)
                    eng.br_lt(r_i, n_iter, "dve_loop", "dve_fin")
                with nc.body("dve_fin"):
                    eng.drain()
                    eng.tensor_tensor(
                        es[:, :], v[:, :], trend[:, :], op=ALU.add
                    ).then_inc(fin_sem, 1)
                    eng.wait_ge(fin_sem, 2)           # ACT exp(es) done
                    eng.drain()
                    eng.tensor_reduce(
                        es[:, 0:1], ln_out[:, :], axis=AX.X, op=ALU.add
                    ).then_inc(fin_sem, 1)
                    eng.wait_ge(fin_sem, 4)           # ACT Ln(psum_f) -> m_sb
                    eng.drain()
                    eng.tensor_tensor(
                        fin[:, :], m_sb[:, :], c_acc[:, :], op=ALU.add
                    ).then_inc(fin_sem, 1)
                    eng.br(block.end_bb)

    nc.compile()
    return nc


_NC_CACHE = {}


def _get_nc(n_steps):
    if n_steps not in _NC_CACHE:
        _NC_CACHE[n_steps] = build_kernel(n_steps)
    return _NC_CACHE[n_steps]


def prep_inputs(h, transitions):
    h = np.ascontiguousarray(np.asarray(h, dtype=np.float32))
    tr = np.ascontiguousarray(np.asarray(transitions, dtype=np.float32))
    n_steps = h.shape[0]
    # p-major tag layout: tag j <-> (p = j // NSLOT, k = j % NSLOT)
    wtb = np.empty((NBLK, P, P), dtype=np.float32)
    for k in range(NSLOT):
        for g in range(NGRP):
            wtb[k * NGRP + g] = tr[g::NGRP, :][:, k::NSLOT].T
    v0 = np.full((T,), -10000.0, dtype=np.float32)
    v0[0] = 0.0
    return {
        "wtb": np.ascontiguousarray(wtb),
        "hsb": h,
        "v0f": np.ascontiguousarray(v0.reshape(P, NSLOT)),
        "trendf": np.ascontiguousarray(tr[1].reshape(P, NSLOT)),
    }


def kernel(h, transitions):
    from concourse.bass_utils import run_bass_kernel_spmd

    inputs = prep_inputs(h, transitions)
    n_steps = inputs["hsb"].shape[0]
    nc = _get_nc(n_steps)
    core_ids = list(range(8))
    in_maps = [dict(inputs) for _ in core_ids]
    res = run_bass_kernel_spmd(nc, in_maps, core_ids)
    return np.asarray(res.results[0]["out"][0, 0], dtype=np.float32)


if __name__ == "__main__":
    import reference

    inputs = {k: np.asarray(v) for k, v in reference.setup_inputs().items()}
    out = kernel(**inputs)
    print("kernel out:", out)

